# revision 9
# baseline (speedup 1.0000x reference)
"""Multi-head attention block (B=32,S=512,D=768,H=12) on 8 TRN2 NeuronCores.

Sharding: data-parallel over batch (4 batches/core), weights replicated,
no collectives. Host pre-transposes x and the weight matrices so the
device kernel is a pure matmul pipeline (no on-chip transposes):

  per core (4 batches), all matmul operands bf16 (host-converted), fp32
  accumulation in PSUM:
    yT[o,t]  = Wqkv xT for q,k rows (o on partitions -> ACT per-partition
             bias during the psum->sbuf copy)
    v[t,:]   stored per head as [1 | pad(63) | v_h(64)] (128 cols/head);
             the leading ones column makes the av matmul put the softmax
             denominator in PSUM row 0 and av_h in rows 64..127.
    per head: scoresT[s,t] = kT^T qT (K=64), exp on ACT ([128,1024] ops,
             scale folded), av in one accumulated matmul (M=128).
             Normalization is per-head and DMA-free on the critical path:
             DVE copies the sums row (PSUM row 0) to SBUF, gpsimd
             broadcasts it to 128 partitions, DVE approx-reciprocal, then
             one DVE multiply reads av straight from PSUM rows 64..127:
             odd heads write avT[c][64:128] in place; even heads write a
             tmp at rows 64:128 which a DMA shifts to avT[c][0:64].
    out[t,:] = avT^T WpT + combo; DVE adds combo during the psum->sbuf
             copy, halves DMA to DRAM as they complete.

Schedule: software-pipelined qkv(b) -> proj(b-1) -> attn(b) with head
staggering (scores(h+1) issue ahead of av(h)), x prefetch and one yT
chunk-pair + v0 of batch b+1 interleaved into the tail of attn(b).
PSUM pools are split (scores / qkv+proj / av) so the next batch's qkv
never waits on attention-normalization drains.
"""

import sys

if "/opt/trn_rl_repo" not in sys.path:
    sys.path.insert(0, "/opt/trn_rl_repo")

from contextlib import ExitStack

import numpy as np

import concourse.tile as tile
from concourse import bacc, mybir
from concourse.bass_utils import run_bass_kernel_spmd

B, S, D = 32, 512, 768
H, HD = 12, 64
SCALE = HD**-0.5
NCORES = 8
NB = B // NCORES  # batches per core
P = 128
TCH = S // P  # token chunks per batch
DCH = D // P  # d chunks
QKC = 2 * D // P  # o-chunks holding q,k
NHALF = D // 2  # 384: N-tile for v/proj matmuls
VW = P  # per-head width in the v tile: [1 | pad 63 | v 64]
F32 = mybir.dt.float32
BF16 = mybir.dt.bfloat16
EXP = mybir.ActivationFunctionType.Exp


def build_nc():
    nc = bacc.Bacc(None, target_bir_lowering=False, debug=False)
    xT = nc.declare_dram_parameter("xT", [NB, D, S], BF16, isOutput=False)
    wqkvT = nc.declare_dram_parameter("wqkvT", [D, 3 * D], BF16, isOutput=False)
    wpT = nc.declare_dram_parameter("wpT", [D, D], BF16, isOutput=False)
    bqkv = nc.declare_dram_parameter("bqkv", [3 * D], F32, isOutput=False)
    combo = nc.declare_dram_parameter("combo", [D], BF16, isOutput=False)
    bv16 = nc.declare_dram_parameter("bv16", [D], BF16, isOutput=False)
    out = nc.declare_dram_parameter("out", [NB, S, D], F32, isOutput=True)

    with ExitStack() as ctx:
        tc = ctx.enter_context(tile.TileContext(nc))
        wp = ctx.enter_context(tc.tile_pool(name="weights", bufs=1))
        sb = ctx.enter_context(tc.tile_pool(name="work", bufs=1))
        ps = ctx.enter_context(tc.tile_pool(name="psum", bufs=1, space="PSUM"))

        # ---- persistent weights / constants ----
        # DMA order: the columns needed by the first two yT chunks (c=0,
        # c=6) land first so the PE can start ~2us earlier; x is split
        # across two queues for the same reason.
        wq_t = [
            wp.tile([P, 3 * D], BF16, name=f"wqkvT{d}", tag=f"wqkvT{d}")
            for d in range(DCH)
        ]
        for d in range(DCH):
            eng = nc.sync if d % 2 == 0 else nc.scalar
            eng.dma_start(out=wq_t[d][:, :P], in_=wqkvT[d * P : (d + 1) * P, :P])
            eng.dma_start(
                out=wq_t[d][:, 6 * P : 7 * P],
                in_=wqkvT[d * P : (d + 1) * P, 6 * P : 7 * P],
            )
        xt0 = []
        for d in range(DCH):
            t = sb.tile([P, S], BF16, name=f"xT_b0_{d}", tag=f"xT{d}", bufs=2)
            eng = nc.gpsimd if d % 2 == 0 else nc.scalar
            eng.dma_start(out=t, in_=xT[0, d * P : (d + 1) * P, :])
            xt0.append(t)
        for d in range(DCH):
            eng = nc.sync if d % 2 == 0 else nc.scalar
            eng.dma_start(
                out=wq_t[d][:, P : 6 * P], in_=wqkvT[d * P : (d + 1) * P, P : 6 * P]
            )
            eng.dma_start(
                out=wq_t[d][:, 7 * P : 2 * D],
                in_=wqkvT[d * P : (d + 1) * P, 7 * P : 2 * D],
            )
        bcols = []
        for c in range(QKC):
            t = wp.tile([P, 1], F32, name=f"bcol{c}", tag=f"bcol{c}")
            nc.sync.dma_start(
                out=t, in_=bqkv[c * P : (c + 1) * P].rearrange("(p o) -> p o", o=1)
            )
            bcols.append(t)
        bvrow = wp.tile([1, D], BF16, name="bvrow", tag="bvrow")
        nc.sync.dma_start(out=bvrow, in_=bv16.rearrange("(o f) -> o f", o=1))
        bvb = wp.tile([P, D], BF16, name="bvb", tag="bvb")
        nc.gpsimd.partition_broadcast(bvb, bvrow)
        for d in range(DCH):
            eng = nc.sync if d % 2 == 0 else nc.scalar
            eng.dma_start(
                out=wq_t[d][:, 2 * D :], in_=wqkvT[d * P : (d + 1) * P, 2 * D :]
            )
        wp_t = []
        for d in range(DCH):
            t = wp.tile([P, D], BF16, name=f"wpT{d}", tag=f"wpT{d}")
            nc.sync.dma_start(out=t, in_=wpT[d * P : (d + 1) * P, :])
            wp_t.append(t)
        comborow = wp.tile([1, D], BF16, name="comborow", tag="comborow")
        nc.sync.dma_start(out=comborow, in_=combo.rearrange("(o f) -> o f", o=1))
        cbb = wp.tile([P, D], BF16, name="cbb", tag="cbb")
        nc.gpsimd.partition_broadcast(cbb, comborow)

        def emit_x_load(b):
            xt = []
            for d in range(DCH):
                t = sb.tile([P, S], BF16, name=f"xT_b{b}_{d}", tag=f"xT{d}", bufs=2)
                eng = nc.gpsimd if d % 2 == 0 else nc.scalar
                eng.dma_start(out=t, in_=xT[b, d * P : (d + 1) * P, :])
                xt.append(t)
            return xt

        def emit_yT_chunk(b, xt, c):
            pt = ps.tile([P, S], F32, name=f"yTps_b{b}_{c}", tag="yv", bufs=2)
            for d in range(DCH):
                nc.tensor.matmul(
                    out=pt,
                    lhsT=wq_t[d][:, c * P : (c + 1) * P],
                    rhs=xt[d],
                    start=(d == 0),
                    stop=(d == DCH - 1),
                )
            st = sb.tile([P, S], BF16, name=f"yT_b{b}_{c}", tag=f"yT{c}", bufs=2)
            nc.scalar.activation(
                st, pt, mybir.ActivationFunctionType.Identity, bias=bcols[c]
            )
            return st

        def emit_v_tile(b, xt, ti):
            # layout per head: [1 | pad(63) | v_h(64)]; pad columns carry
            # junk (their psum rows are never read)
            vtile = sb.tile(
                [P, H * VW], BF16, name=f"v_b{b}_{ti}", tag=f"v{ti}", bufs=2
            )
            nc.vector.memset(
                vtile.rearrange("p (h k) -> p h k", k=VW)[:, :, 0:1], 1.0
            )
            for half in range(2):
                pv = ps.tile(
                    [P, NHALF], F32, name=f"vps_b{b}_{ti}_{half}", tag="av", bufs=2
                )
                o0 = 2 * D + half * NHALF
                for d in range(DCH):
                    nc.tensor.matmul(
                        out=pv,
                        lhsT=xt[d][:, ti * P : (ti + 1) * P],
                        rhs=wq_t[d][:, o0 : o0 + NHALF],
                        start=(d == 0),
                        stop=(d == DCH - 1),
                    )
                nc.vector.tensor_tensor(
                    out=vtile.rearrange("p (h k) -> p h k", k=VW)[
                        :, 6 * half : 6 * (half + 1), HD:VW
                    ],
                    in0=pv.rearrange("p (h k) -> p h k", k=HD),
                    in1=bvb[:, half * NHALF : (half + 1) * NHALF].rearrange(
                        "p (h k) -> p h k", k=HD
                    ),
                    op=mybir.AluOpType.add,
                )
            return vtile

        def emit_scores(b, h, yt):
            hp = (h % 2) * HD
            qs = yt[h // 2][hp : hp + HD, :]
            ks = yt[6 + h // 2][hp : hp + HD, :]
            exps = []
            for jp in range(2):
                pt = ps.tile(
                    [P, 2 * S], F32, name=f"sc_b{b}_h{h}_{jp}", tag="sc", bufs=2
                )
                for jj in range(2):
                    j = 2 * jp + jj
                    nc.tensor.matmul(
                        out=pt[:, jj * S : (jj + 1) * S],
                        lhsT=ks[:, j * P : (j + 1) * P],
                        rhs=qs,
                        start=True,
                        stop=True,
                    )
                et = sb.tile(
                    [P, 2 * S], BF16, name=f"expT_b{b}_h{h}_{jp}", tag="expT",
                    bufs=4,
                )
                nc.scalar.activation(et, pt, EXP, scale=SCALE)
                exps.append(et)
            return exps

        def emit_av(b, h, exps, vt, avt):
            c = h // 2
            pav = ps.tile([P, S], F32, name=f"av_b{b}_h{h}", tag="av", bufs=2)
            for j in range(TCH):
                nc.tensor.matmul(
                    out=pav,
                    lhsT=vt[j][:, h * VW : (h + 1) * VW],
                    rhs=exps[j // 2][:, (j % 2) * S : (j % 2 + 1) * S],
                    start=(j == 0),
                    stop=(j == TCH - 1),
                )
            # denominator: PSUM row 0 -> SBUF p0 -> broadcast -> reciprocal
            ssb = sb.tile([1, S], F32, name=f"ssb_b{b}_h{h}", tag="ssb", bufs=4)
            nc.vector.tensor_copy(ssb, pav[0:1, :])
            bcs = sb.tile([P, S], F32, name=f"bcs_b{b}_h{h}", tag="bcs", bufs=2)
            nc.gpsimd.partition_broadcast(bcs, ssb)
            bcr = sb.tile([P, S], F32, name=f"bcr_b{b}_h{h}", tag="bcr", bufs=2)
            nc.vector.reciprocal_approx_fast(bcr, bcs)
            if h % 2 == 1:
                nc.vector.tensor_tensor(
                    out=avt[c][HD:P, :],
                    in0=pav[HD:P, :],
                    in1=bcr[HD:P, :],
                    op=mybir.AluOpType.mult,
                )
            else:
                tmp = sb.tile([P, S], BF16, name=f"avtmp_b{b}_h{h}", tag="avtmp",
                              bufs=2)
                nc.vector.tensor_tensor(
                    out=tmp[HD:P, :],
                    in0=pav[HD:P, :],
                    in1=bcr[HD:P, :],
                    op=mybir.AluOpType.mult,
                )
                nc.gpsimd.dma_start(out=avt[c][0:HD, :], in_=tmp[HD:P, :])

        def emit_proj(b, avt):
            # two waves of 4 interleaved accumulation groups
            tags = ["sc", "sc", "yv", "yv"]
            fts = {}
            for wave_t in range(2):
                groups = []
                for k in range(4):
                    ti = 2 * wave_t + (k // 2)
                    half = k % 2
                    pf = ps.tile(
                        [P, NHALF], F32, name=f"fps_b{b}_{ti}_{half}",
                        tag=tags[k], bufs=2,
                    )
                    groups.append((pf, ti, half))
                for pf, ti, half in groups:
                    for d in range(4):
                        nc.tensor.matmul(
                            out=pf,
                            lhsT=avt[d][:, ti * P : (ti + 1) * P],
                            rhs=wp_t[d][:, half * NHALF : (half + 1) * NHALF],
                            start=(d == 0),
                            stop=False,
                        )
                for pf, ti, half in groups:
                    for d in range(4, DCH):
                        nc.tensor.matmul(
                            out=pf,
                            lhsT=avt[d][:, ti * P : (ti + 1) * P],
                            rhs=wp_t[d][:, half * NHALF : (half + 1) * NHALF],
                            start=False,
                            stop=(d == DCH - 1),
                        )
                    if ti not in fts:
                        fts[ti] = sb.tile(
                            [P, D], F32, name=f"fin_b{b}_{ti}", tag="fin", bufs=3
                        )
                    nc.vector.tensor_tensor(
                        out=fts[ti][:, half * NHALF : (half + 1) * NHALF],
                        in0=pf,
                        in1=cbb[:, half * NHALF : (half + 1) * NHALF],
                        op=mybir.AluOpType.add,
                    )
                    nc.sync.dma_start(
                        out=out[
                            b,
                            ti * P : (ti + 1) * P,
                            half * NHALF : (half + 1) * NHALF,
                        ],
                        in_=fts[ti][:, half * NHALF : (half + 1) * NHALF],
                    )

        # ---- main schedule: qkv(b) -> proj(b-1) -> attn(b) ----
        prev = None
        pre = {}
        xt = xt0
        for b in range(NB):
            yt = [None] * QKC
            for hp in range(6):
                for c in (hp, 6 + hp):
                    yt[c] = pre[c] if c in pre else emit_yT_chunk(b, xt, c)
            vt = [
                pre["v0"] if ti == 0 and "v0" in pre else emit_v_tile(b, xt, ti)
                for ti in range(TCH)
            ]
            if prev is not None:
                emit_proj(b - 1, prev)
            if b + 1 < NB:
                xt = emit_x_load(b + 1)
            avt = [
                sb.tile([P, S], BF16, name=f"avT_b{b}_{c}", tag=f"avT{c}", bufs=2)
                for c in range(DCH)
            ]
            pre_next = {}
            prev_exps = None
            for h in range(H):
                cur_exps = emit_scores(b, h, yt)
                if h > 0:
                    emit_av(b, h - 1, prev_exps, vt, avt)
                prev_exps = cur_exps
                if h == 10 and b + 1 < NB:
                    pre_next[0] = emit_yT_chunk(b + 1, xt, 0)
                    pre_next[6] = emit_yT_chunk(b + 1, xt, 6)
            emit_av(b, H - 1, prev_exps, vt, avt)
            if b + 1 < NB:
                pre_next["v0"] = emit_v_tile(b + 1, xt, 0)
            pre = pre_next
            prev = avt
        emit_proj(NB - 1, prev)

    nc.compile()
    return nc


_CACHE = {}


def _get_nc():
    if "nc" not in _CACHE:
        _CACHE["nc"] = build_nc()
    return _CACHE["nc"]


def _prepare_in_maps(x, qkv_w, qkv_b, proj_w, proj_b):
    x = np.asarray(x, dtype=np.float32)
    qkv_w = np.asarray(qkv_w, dtype=np.float32)
    qkv_b = np.asarray(qkv_b, dtype=np.float32)
    proj_w = np.asarray(proj_w, dtype=np.float32)
    proj_b = np.asarray(proj_b, dtype=np.float32)
    import ml_dtypes

    bf16 = ml_dtypes.bfloat16
    wqkvT = np.ascontiguousarray(qkv_w.T).astype(bf16)
    wpT = np.ascontiguousarray(proj_w.T).astype(bf16)
    combo = proj_b.astype(bf16)  # v-bias flows through softmax via bvrow
    bv16 = qkv_b[2 * D :].astype(bf16)
    in_maps = []
    for c in range(NCORES):
        xs = x[c * NB : (c + 1) * NB]
        xTs = np.ascontiguousarray(xs.transpose(0, 2, 1)).astype(bf16)
        in_maps.append(
            {
                "xT": xTs,
                "wqkvT": wqkvT,
                "wpT": wpT,
                "bqkv": qkv_b,
                "combo": combo,
                "bv16": bv16,
            }
        )
    return in_maps


def kernel(x, qkv_w, qkv_b, proj_w, proj_b):
    nc = _get_nc()
    in_maps = _prepare_in_maps(x, qkv_w, qkv_b, proj_w, proj_b)
    res = run_bass_kernel_spmd(nc, in_maps, core_ids=list(range(NCORES)))
    return np.concatenate([res.results[i]["out"] for i in range(NCORES)], axis=0)
